# revision 3
# baseline (speedup 1.0000x reference)
"""HandNet GCN kernel for 8x Trainium2 NeuronCores (Bass/Tile).

Network (per batch b of 8192, N=21 hand joints):
  x1 = relu(A @ x  @ W1 + b1)   [21,256] -> [21,128]
  x2 = relu(A @ x1 @ W2 + b2)   [21,128] -> [21,128]
  x3 = relu(A @ x2 @ W3 + b3)   [21,128] -> [21,512]
  out = x3 @ fcW + fcb          [21,512] -> [21,3]
Returns (x3, out).

Strategy: pure data parallelism over batch (1024 batches/core, padded to
1056). Six batches are packed per partition-group (6*21 = 126 rows), and the
adjacency is applied as kron(I6, A) [126x126]. Each layer runs as two
matmuls with alternating operand roles so the activation transpose is free:
  A-mix : lhsT = act(node-major) [126, Fc], rhs = A6 [126,126]
          -> out = (A @ X)^T feature-major in PSUM
  W-mul : lhsT = P(feature-major) [128, 126], rhs = W chunk -> node-major,
          or with W3 chunks as lhsT -> feature-major.
x3 and out are produced feature-major and transposed on the host.
"""

import numpy as np

NUM_HAND = 21
PACK = 6                      # batches per partition group
P_ROWS = PACK * NUM_HAND      # 126
N_CORES = 8
B_FULL = 8192
B_CORE = B_FULL // N_CORES    # 1024
GROUPS = 176                  # per core, multiple of 8 (=> B_PAD = 1056)
B_PAD = GROUPS * PACK         # 1056
ROWS_PAD = B_PAD * NUM_HAND   # 22176
F_IN = 256
D_MODEL = 128
F_OUT = 512
OUT_DIM = 3

BLK = 8                       # groups per outer block (x3/out DMA unit)
NBLK = GROUPS // BLK          # 22
XIN_G = 4                     # groups per input DMA

_CACHE = {}


def _build_program(has_b1, has_b2, has_b3, has_fcb):
    import concourse.bacc as bacc
    import concourse.mybir as mybir
    from concourse.tile import TileContext

    f32 = mybir.dt.float32
    nc = bacc.Bacc("TRN2")

    x_d = nc.dram_tensor("x", [ROWS_PAD, F_IN], f32, kind="ExternalInput")
    a6_d = nc.dram_tensor("a6", [3, P_ROWS, P_ROWS], f32, kind="ExternalInput")
    w1_d = nc.dram_tensor("w1p", [D_MODEL, F_IN], f32, kind="ExternalInput")
    w2_d = nc.dram_tensor("w2", [D_MODEL, D_MODEL], f32, kind="ExternalInput")
    w3_d = nc.dram_tensor("w3", [D_MODEL, F_OUT], f32, kind="ExternalInput")
    fcw_d = nc.dram_tensor("fcwp", [D_MODEL, 4 * OUT_DIM], f32, kind="ExternalInput")
    x3_d = nc.dram_tensor("x3fm", [F_OUT, ROWS_PAD], f32, kind="ExternalOutput")
    out_d = nc.dram_tensor("outfm", [OUT_DIM, ROWS_PAD], f32, kind="ExternalOutput")

    b1_d = b2_d = b3_d = fcb_d = None
    if has_b1:
        b1_d = nc.dram_tensor("b1r", [P_ROWS, 2 * D_MODEL], f32, kind="ExternalInput")
    if has_b2:
        b2_d = nc.dram_tensor("b2r", [P_ROWS, 2 * D_MODEL], f32, kind="ExternalInput")
    if has_b3:
        b3_d = nc.dram_tensor("b3r", [D_MODEL, 4, P_ROWS], f32, kind="ExternalInput")
    if has_fcb:
        fcb_d = nc.dram_tensor("fcbr", [OUT_DIM, 1], f32, kind="ExternalInput")

    Relu = mybir.ActivationFunctionType.Relu
    Add = mybir.AluOpType.add

    # DRAM views
    # x rows grouped: [NBLK*2, 126, XIN_G, 256] -- XIN_G groups per DMA
    x_view = x_d.rearrange("(nb gb p) f -> nb p gb f", gb=XIN_G, p=P_ROWS)
    # x3 fm: row (c*128+f), col (blk, go, bn); per blk: [128, 4, 1008]
    x3_view = x3_d.rearrange("(c f) (nb w) -> nb f c w", f=D_MODEL, w=BLK * P_ROWS)

    with TileContext(nc) as tc:
        with (
            tc.tile_pool(name="consts", bufs=1) as consts,
            tc.tile_pool(name="xin", bufs=3) as xin_pool,
            tc.tile_pool(name="stage", bufs=3) as stage,
            tc.tile_pool(name="x3s", bufs=2) as x3s_pool,
            tc.tile_pool(name="outs", bufs=2) as outs_pool,
            tc.tile_pool(name="pP", bufs=3, space="PSUM") as pP,
            tc.tile_pool(name="pY", bufs=2, space="PSUM") as pY,
            tc.tile_pool(name="pX3", bufs=2, space="PSUM") as pX3,
            tc.tile_pool(name="pOut", bufs=1, space="PSUM") as pOut,
        ):
            # ---- constants ----
            a6 = []
            for l in range(3):
                t = consts.tile([P_ROWS, P_ROWS], f32, tag=f"a6_{l}")
                nc.sync.dma_start(out=t, in_=a6_d[l])
                a6.append(t)
            w1 = consts.tile([D_MODEL, F_IN], f32, tag="w1")
            nc.sync.dma_start(out=w1, in_=w1_d[:])
            w2 = consts.tile([D_MODEL, D_MODEL], f32, tag="w2")
            nc.sync.dma_start(out=w2, in_=w2_d[:])
            w3 = consts.tile([D_MODEL, F_OUT], f32, tag="w3")
            nc.sync.dma_start(out=w3, in_=w3_d[:])
            fcw = consts.tile([D_MODEL, 4 * OUT_DIM], f32, tag="fcw")
            nc.sync.dma_start(out=fcw, in_=fcw_d[:])
            b1r = b2r = b3r = fcbr = None
            if has_b1:
                b1r = consts.tile([P_ROWS, 2 * D_MODEL], f32, tag="b1r")
                nc.sync.dma_start(out=b1r, in_=b1_d[:])
            if has_b2:
                b2r = consts.tile([P_ROWS, 2 * D_MODEL], f32, tag="b2r")
                nc.sync.dma_start(out=b2r, in_=b2_d[:])
            if has_b3:
                b3r = consts.tile([D_MODEL, 4, P_ROWS], f32, tag="b3r")
                nc.sync.dma_start(out=b3r, in_=b3_d[:])
            if has_fcb:
                fcbr = consts.tile([OUT_DIM, 1], f32, tag="fcbr")
                nc.sync.dma_start(out=fcbr, in_=fcb_d[:])

            def relu_evict_nm(dst, src, brep):
                """dst = relu(src + bias); node-major [126, 2, 128]."""
                if brep is None:
                    nc.vector.tensor_relu(dst, src)
                else:
                    nc.vector.tensor_tensor(dst, src, brep, Add)
                    nc.scalar.activation(dst, dst, Relu)

            # ---- main loop ----
            for blk in range(NBLK):
                # input: 2 DMAs of 4 groups each
                xts = []
                for h in range(2):
                    xt = xin_pool.tile([P_ROWS, XIN_G, F_IN], f32, tag="xt")
                    nc.sync.dma_start(out=xt, in_=x_view[2 * blk + h])
                    xts.append(xt)
                # x3 staging for this blk: [128, c=4, go=8, bn=126]
                x3sb = x3s_pool.tile([D_MODEL, 4, BLK, P_ROWS], f32, tag="x3sb")
                outsb = outs_pool.tile([OUT_DIM, BLK, P_ROWS], f32, tag="outsb")
                pout = None

                for sg in range(BLK // 2):  # supergroups of 2 groups
                    # -------- L1 A-mix: 2 groups x 2 chunks --> p1 fm
                    p1 = pP.tile([D_MODEL, 4, P_ROWS], f32, tag="pP")
                    for g2 in range(2):
                        go = sg * 2 + g2
                        xsl = xts[go // XIN_G][:, go % XIN_G]
                        for c in range(2):
                            nc.tensor.matmul(
                                p1[:, g2 * 2 + c],
                                xsl[:, c * D_MODEL:(c + 1) * D_MODEL],
                                a6[0],
                            )
                    p1s = stage.tile([D_MODEL, 4, P_ROWS], f32, tag="p1s")
                    nc.scalar.copy(p1s, p1)
                    # -------- L1 W-mul: fm lhsT, W1 rhs --> Y1 node-major
                    y1 = pY.tile([P_ROWS, 2, D_MODEL], f32, tag="pY")
                    for g2 in range(2):
                        for c in range(2):
                            nc.tensor.matmul(
                                y1[:, g2],
                                p1s[:, g2 * 2 + c],
                                w1[:, c * D_MODEL:(c + 1) * D_MODEL],
                                start=(c == 0),
                                stop=(c == 1),
                            )
                    h2 = stage.tile([P_ROWS, 2, D_MODEL], f32, tag="h2")
                    relu_evict_nm(h2, y1, b1r)
                    # -------- L2
                    p2 = pP.tile([D_MODEL, 2, P_ROWS], f32, tag="pP")
                    for g2 in range(2):
                        nc.tensor.matmul(p2[:, g2], h2[:, g2], a6[1])
                    p2s = stage.tile([D_MODEL, 2, P_ROWS], f32, tag="p2s")
                    nc.scalar.copy(p2s, p2)
                    y2 = pY.tile([P_ROWS, 2, D_MODEL], f32, tag="pY")
                    for g2 in range(2):
                        nc.tensor.matmul(y2[:, g2], p2s[:, g2], w2)
                    h3 = stage.tile([P_ROWS, 2, D_MODEL], f32, tag="h3")
                    relu_evict_nm(h3, y2, b2r)
                    # -------- L3 A-mix
                    p3 = pP.tile([D_MODEL, 2, P_ROWS], f32, tag="pP")
                    for g2 in range(2):
                        nc.tensor.matmul(p3[:, g2], h3[:, g2], a6[2])
                    p3s = stage.tile([D_MODEL, 2, P_ROWS], f32, tag="p3s")
                    nc.scalar.copy(p3s, p3)
                    # -------- L3 W-mul feature-major: W3 chunk lhsT
                    for g2 in range(2):
                        go = sg * 2 + g2
                        px3 = pX3.tile([D_MODEL, 4, P_ROWS], f32, tag="pX3")
                        for c in range(4):
                            nc.tensor.matmul(
                                px3[:, c],
                                w3[:, c * D_MODEL:(c + 1) * D_MODEL],
                                p3s[:, g2],
                            )
                        # evict: x3sb[:, :, go, :] = relu(px3 + b3)
                        dst = x3sb[:, :, go, :]
                        if b3r is None:
                            if g2 == 0:
                                nc.scalar.activation(dst, px3, Relu)
                            else:
                                nc.vector.tensor_relu(dst, px3)
                        else:
                            nc.vector.tensor_tensor(dst, px3, b3r, Add)
                            nc.scalar.activation(dst, dst, Relu)
                        # -------- FC: fcW chunks as lhsT (M=3), accumulate
                        gi = go % 4
                        if gi == 0:
                            pout = pOut.tile([OUT_DIM, 4, P_ROWS], f32, tag="pOut")
                        for c in range(4):
                            nc.tensor.matmul(
                                pout[:, gi],
                                fcw[:, c * OUT_DIM:(c + 1) * OUT_DIM],
                                x3sb[:, c, go, :],
                                start=(c == 0),
                                stop=(c == 3),
                            )
                        if gi == 3:
                            dst = outsb[:, go - 3:go + 1, :]
                            if fcbr is None:
                                nc.vector.tensor_copy(dst, pout)
                            else:
                                nc.vector.tensor_scalar_add(dst, pout, fcbr)

                # blk outputs
                nc.sync.dma_start(
                    out=x3_view[blk],
                    in_=x3sb.rearrange("f c go bn -> f c (go bn)"),
                )
                nc.sync.dma_start(
                    out=out_d[:, blk * BLK * P_ROWS:(blk + 1) * BLK * P_ROWS],
                    in_=outsb.rearrange("o g bn -> o (g bn)"),
                )
    nc.compile()
    return nc


def _get_program(key):
    if key not in _CACHE:
        _CACHE[key] = _build_program(*key)
    return _CACHE[key]


def _host_inputs(x, A1, A2, A3, W1, b1, W2, b2, W3, b3, fcW, fcb):
    """Build per-core input maps. Returns (in_maps, key)."""
    A1, A2, A3 = (np.asarray(a, np.float32) for a in (A1, A2, A3))
    W1, W2, W3, fcW = (np.asarray(w, np.float32) for w in (W1, W2, W3, fcW))
    b1, b2, b3, fcb = (np.asarray(b, np.float32) for b in (b1, b2, b3, fcb))

    eye6 = np.eye(PACK, dtype=np.float32)
    a6 = np.ascontiguousarray(
        np.stack([np.kron(eye6, a) for a in (A1, A2, A3)]))  # [3,126,126]
    # W1 packed: [128, 256], chunk c cols = W1[c*128:(c+1)*128, :]
    w1p = np.ascontiguousarray(
        np.concatenate([W1[0:128, :], W1[128:256, :]], axis=1))
    w2 = np.ascontiguousarray(W2)
    w3 = np.ascontiguousarray(W3)
    fcwp = np.ascontiguousarray(
        np.concatenate([fcW[c * 128:(c + 1) * 128, :] for c in range(4)], axis=1))

    has_b1, has_b2 = bool(b1.any()), bool(b2.any())
    has_b3, has_fcb = bool(b3.any()), bool(fcb.any())
    key = (has_b1, has_b2, has_b3, has_fcb)

    extra = {}
    if has_b1:
        extra["b1r"] = np.ascontiguousarray(
            np.broadcast_to(np.tile(b1, 2)[None, :], (P_ROWS, 2 * D_MODEL)))
    if has_b2:
        extra["b2r"] = np.ascontiguousarray(
            np.broadcast_to(np.tile(b2, 2)[None, :], (P_ROWS, 2 * D_MODEL)))
    if has_b3:
        # b3r[f, c, bn] = b3[c*128+f]
        b3r = np.empty((D_MODEL, 4, P_ROWS), np.float32)
        for c in range(4):
            b3r[:, c, :] = b3[c * 128:(c + 1) * 128][:, None]
        extra["b3r"] = b3r
    if has_fcb:
        extra["fcbr"] = np.ascontiguousarray(fcb[:, None])

    xs = np.asarray(x, np.float32).reshape(B_FULL, NUM_HAND, F_IN)
    in_maps = []
    for c in range(N_CORES):
        shard = xs[c * B_CORE:(c + 1) * B_CORE]
        shard = np.pad(shard, ((0, B_PAD - B_CORE), (0, 0), (0, 0)))
        m = {
            "x": np.ascontiguousarray(shard.reshape(ROWS_PAD, F_IN)),
            "a6": a6, "w1p": w1p, "w2": w2, "w3": w3, "fcwp": fcwp,
        }
        m.update(extra)
        in_maps.append(m)
    return in_maps, key


def _run_cores(x, A1, A2, A3, W1, b1, W2, b2, W3, b3, fcW, fcb,
               run_fn, **run_kwargs):
    in_maps, key = _host_inputs(x, A1, A2, A3, W1, b1, W2, b2, W3, b3, fcW, fcb)
    nc = _get_program(key)
    res = run_fn(nc, in_maps, core_ids=list(range(N_CORES)), **run_kwargs)
    results = res.results if hasattr(res, "results") else res
    x3_parts, out_parts = [], []
    nrows = B_CORE * NUM_HAND
    for c in range(N_CORES):
        x3fm = np.asarray(results[c]["x3fm"])[:, :nrows]
        outfm = np.asarray(results[c]["outfm"])[:, :nrows]
        x3_parts.append(np.ascontiguousarray(x3fm.T).reshape(
            B_CORE, NUM_HAND, F_OUT))
        out_parts.append(np.ascontiguousarray(outfm.T).reshape(
            B_CORE, NUM_HAND, OUT_DIM))
    x3 = np.concatenate(x3_parts, axis=0)
    out = np.concatenate(out_parts, axis=0)
    return (x3, out)


def kernel(x, A1, A2, A3, W1, b1, W2, b2, W3, b3, fcW, fcb):
    from concourse.bass_utils import run_bass_kernel_spmd
    return _run_cores(x, A1, A2, A3, W1, b1, W2, b2, W3, b3, fcW, fcb,
                      run_bass_kernel_spmd)


# revision 5
# speedup vs baseline: 2.2792x; 2.2792x over previous
"""HandNet GCN kernel for 8x Trainium2 NeuronCores (Bass/Tile).

Network (per batch b of 8192, N=21 hand joints):
  x1 = relu(A @ x  @ W1 + b1)   [21,256] -> [21,128]
  x2 = relu(A @ x1 @ W2 + b2)   [21,128] -> [21,128]
  x3 = relu(A @ x2 @ W3 + b3)   [21,128] -> [21,512]
  out = x3 @ fcW + fcb          [21,512] -> [21,3]
Returns (x3, out).

Strategy: pure data parallelism over batch (1024 batches/core, padded to
1056). Six batches are packed per partition-group (6*21 = 126 rows), and
the adjacency is applied as kron(I6, A) [126x126]. Each layer runs as two
matmul families with alternating operand roles so the activation transpose
rides on LDWEIGHTS:
  A-mix : lhsT = act(node-major) [126, 128], rhs = A6 [126,126]
          -> out = (A @ X)^T feature-major in PSUM
  W-mul : lhsT = P(feature-major) [128, 128], rhs = W chunk -> node-major
  L3/FC : W3 chunks / fcW chunks as lhsT, activations as wide moving
          operand (N=504..512) -> feature-major.
Matmul operands are fp16 (fp32 PSUM accumulate); the final FC runs in fp32
off the fp32 x3 tiles. x3/out are produced feature-major and transposed on
the host; x is pre-cast to fp16 on the host (halves input DMA traffic).
"""

import numpy as np

NUM_HAND = 21
PACK = 6                      # batches per partition group
P_ROWS = PACK * NUM_HAND      # 126
N_CORES = 8
B_FULL = 8192
B_CORE = B_FULL // N_CORES    # 1024
GROUPS = 176                  # per core, multiple of 8 (=> B_PAD = 1056)
B_PAD = GROUPS * PACK         # 1056
ROWS_PAD = B_PAD * NUM_HAND   # 22176
F_IN = 256
D_MODEL = 128
F_OUT = 512
OUT_DIM = 3

BLK = 8                       # groups per outer block (x3/out DMA unit)
NBLK = GROUPS // BLK          # 22
QUAD = 4                      # groups per quad (inner compute unit)
W504 = QUAD * P_ROWS          # 504

COMPUTE_FP16 = True           # fp16 matmul operands (fp32 accumulate)

_CACHE = {}


def _build_program(has_b1, has_b2, has_b3, has_fcb):
    import concourse.bacc as bacc
    import concourse.mybir as mybir
    from concourse.tile import TileContext

    f32 = mybir.dt.float32
    cdt = mybir.dt.float16 if COMPUTE_FP16 else f32
    nc = bacc.Bacc("TRN2")

    x_d = nc.dram_tensor("x", [ROWS_PAD, F_IN], cdt, kind="ExternalInput")
    a6_d = nc.dram_tensor("a6", [3, P_ROWS, P_ROWS], cdt, kind="ExternalInput")
    w1_d = nc.dram_tensor("w1p", [D_MODEL, F_IN], cdt, kind="ExternalInput")
    w2_d = nc.dram_tensor("w2", [D_MODEL, D_MODEL], cdt, kind="ExternalInput")
    w3_d = nc.dram_tensor("w3", [D_MODEL, F_OUT], cdt, kind="ExternalInput")
    fcw_d = nc.dram_tensor("fcwp", [D_MODEL, 4 * OUT_DIM], f32, kind="ExternalInput")
    x3_d = nc.dram_tensor("x3fm", [F_OUT, ROWS_PAD], f32, kind="ExternalOutput")
    out_d = nc.dram_tensor("outfm", [OUT_DIM, ROWS_PAD], f32, kind="ExternalOutput")

    b1_d = b2_d = b3_d = fcb_d = None
    if has_b1:
        b1_d = nc.dram_tensor("b1r", [P_ROWS, D_MODEL], f32, kind="ExternalInput")
    if has_b2:
        b2_d = nc.dram_tensor("b2r", [P_ROWS, D_MODEL], f32, kind="ExternalInput")
    if has_b3:
        b3_d = nc.dram_tensor("b3r", [D_MODEL, 4], f32, kind="ExternalInput")
    if has_fcb:
        fcb_d = nc.dram_tensor("fcbr", [OUT_DIM, 1], f32, kind="ExternalInput")

    Relu = mybir.ActivationFunctionType.Relu
    Add = mybir.AluOpType.add

    # x rows grouped per blk: [NBLK, 126, BLK, 256]
    x_view = x_d.rearrange("(nb gb p) f -> nb p gb f", gb=BLK, p=P_ROWS)
    # x3 fm: row (c*128+f), col (blk, go, bn); per blk: [128, 4, 1008]
    x3_view = x3_d.rearrange("(c f) (nb w) -> nb f c w", f=D_MODEL, w=BLK * P_ROWS)

    with TileContext(nc) as tc:
        with (
            tc.tile_pool(name="consts", bufs=1) as consts,
            tc.tile_pool(name="xin", bufs=3) as xin_pool,
            tc.tile_pool(name="stage", bufs=2) as stage,
            tc.tile_pool(name="x3s", bufs=2) as x3s_pool,
            tc.tile_pool(name="outs", bufs=2) as outs_pool,
            tc.tile_pool(name="pP1", bufs=1, space="PSUM") as pP1,
            tc.tile_pool(name="pYP", bufs=3, space="PSUM") as pYP,
            tc.tile_pool(name="pX3", bufs=2, space="PSUM") as pX3,
            tc.tile_pool(name="pOut", bufs=1, space="PSUM") as pOut,
        ):
            # ---- constants ----
            a6 = []
            for l in range(3):
                t = consts.tile([P_ROWS, P_ROWS], cdt, tag=f"a6_{l}")
                nc.sync.dma_start(out=t, in_=a6_d[l])
                a6.append(t)
            w1 = consts.tile([D_MODEL, F_IN], cdt, tag="w1")
            nc.sync.dma_start(out=w1, in_=w1_d[:])
            w2 = consts.tile([D_MODEL, D_MODEL], cdt, tag="w2")
            nc.sync.dma_start(out=w2, in_=w2_d[:])
            w3 = consts.tile([D_MODEL, F_OUT], cdt, tag="w3")
            nc.sync.dma_start(out=w3, in_=w3_d[:])
            fcw = consts.tile([D_MODEL, 4 * OUT_DIM], f32, tag="fcw")
            nc.sync.dma_start(out=fcw, in_=fcw_d[:])
            b1r = b2r = b3r = fcbr = None
            if has_b1:
                b1r = consts.tile([P_ROWS, D_MODEL], f32, tag="b1r")
                nc.sync.dma_start(out=b1r, in_=b1_d[:])
            if has_b2:
                b2r = consts.tile([P_ROWS, D_MODEL], f32, tag="b2r")
                nc.sync.dma_start(out=b2r, in_=b2_d[:])
            if has_b3:
                b3r = consts.tile([D_MODEL, 4], f32, tag="b3r")
                nc.sync.dma_start(out=b3r, in_=b3_d[:])
            if has_fcb:
                fcbr = consts.tile([OUT_DIM, 1], f32, tag="fcbr")
                nc.sync.dma_start(out=fcbr, in_=fcb_d[:])

            def relu_evict_nm(use_act, dst, src, brep):
                """dst = relu(src + b); node-major [126,4,128]; bias on free."""
                if brep is None:
                    if use_act:
                        nc.scalar.activation(dst, src, Relu)
                    else:
                        nc.vector.tensor_relu(dst, src)
                else:
                    for g in range(QUAD):
                        nc.vector.tensor_tensor(dst[:, g], src[:, g], brep, Add)
                    nc.scalar.activation(dst, dst, Relu)

            # ---- main loop ----
            for blk in range(NBLK):
                xt = xin_pool.tile([P_ROWS, BLK, F_IN], cdt, tag="xt")
                nc.sync.dma_start(out=xt, in_=x_view[blk])
                # x3 staging: [128, c=4, q=2, (g bn)=504] f32
                x3sb = x3s_pool.tile([D_MODEL, 4, 2, W504], f32, tag="x3sb")
                outsb = outs_pool.tile([OUT_DIM, BLK * P_ROWS], f32, tag="outsb")

                for q in range(2):  # quads of 4 groups
                    # ---- L1 A-mix: 4 groups x 2 chunks -> p1 fm
                    p1 = pP1.tile([D_MODEL, 8, D_MODEL], f32, tag="pP1")
                    for g in range(QUAD):
                        xsl = xt[:, q * QUAD + g]
                        for c in range(2):
                            nc.tensor.matmul(
                                p1[:, g * 2 + c, :P_ROWS],
                                xsl[:, c * D_MODEL:(c + 1) * D_MODEL],
                                a6[0],
                            )
                    p1s = stage.tile([D_MODEL, 8, D_MODEL], cdt, tag="p1s")
                    nc.scalar.copy(p1s, p1)
                    # ---- L1 W-mul -> y1 node-major
                    y1 = pYP.tile([D_MODEL, QUAD, D_MODEL], f32, tag="pYP")
                    for g in range(QUAD):
                        for c in range(2):
                            nc.tensor.matmul(
                                y1[:, g],
                                p1s[:, g * 2 + c],
                                w1[:, c * D_MODEL:(c + 1) * D_MODEL],
                                start=(c == 0),
                                stop=(c == 1),
                            )
                    h2 = stage.tile([P_ROWS, QUAD, D_MODEL], cdt, tag="h2")
                    relu_evict_nm(True, h2, y1[:P_ROWS], b1r)
                    # ---- L2
                    p2 = pYP.tile([D_MODEL, QUAD, D_MODEL], f32, tag="pYP")
                    for g in range(QUAD):
                        nc.tensor.matmul(p2[:, g, :P_ROWS], h2[:, g], a6[1])
                    p2s = stage.tile([D_MODEL, QUAD, D_MODEL], cdt, tag="p2s")
                    nc.scalar.copy(p2s, p2)
                    y2 = pYP.tile([D_MODEL, QUAD, D_MODEL], f32, tag="pYP")
                    for g in range(QUAD):
                        nc.tensor.matmul(y2[:, g], p2s[:, g], w2)
                    h3 = stage.tile([P_ROWS, QUAD, D_MODEL], cdt, tag="h3")
                    relu_evict_nm(False, h3, y2[:P_ROWS], b2r)
                    # ---- L3 A-mix
                    p3 = pYP.tile([D_MODEL, QUAD, D_MODEL], f32, tag="pYP")
                    for g in range(QUAD):
                        nc.tensor.matmul(p3[:, g, :P_ROWS], h3[:, g], a6[2])
                    p3s = stage.tile([D_MODEL, QUAD, D_MODEL], cdt, tag="p3s")
                    nc.vector.tensor_copy(p3s, p3)
                    # ---- L3 W-mul fm, one wide matmul per W3 chunk
                    p3w = p3s.rearrange("f g d -> f (g d)")  # [128, 512]
                    for c in range(4):
                        px3 = pX3.tile([D_MODEL, QUAD, D_MODEL], f32, tag="pX3")
                        nc.tensor.matmul(
                            px3.rearrange("f g d -> f (g d)"),
                            w3[:, c * D_MODEL:(c + 1) * D_MODEL],
                            p3w,
                        )
                        # evict valid cols: [128, 4, :126] -> x3sb[:, c, q, :]
                        dst = x3sb[:, c, q].rearrange("f (g bn) -> f g bn",
                                                      g=QUAD)
                        src = px3[:, :, :P_ROWS]
                        if b3r is None:
                            if c % 2 == 0:
                                nc.scalar.activation(dst, src, Relu)
                            else:
                                nc.vector.tensor_relu(dst, src)
                        else:
                            nc.scalar.activation(dst, src, Relu,
                                                 bias=b3r[:, c:c + 1])
                    # ---- FC (fp32): fcW chunks as lhsT, accumulate over c
                    pout = pOut.tile([OUT_DIM, W504], f32, tag="pOut")
                    for c in range(4):
                        nc.tensor.matmul(
                            pout,
                            fcw[:, c * OUT_DIM:(c + 1) * OUT_DIM],
                            x3sb[:, c, q],
                            start=(c == 0),
                            stop=(c == 3),
                        )
                    dst = outsb[:, q * W504:(q + 1) * W504]
                    if fcbr is None:
                        nc.vector.tensor_copy(dst, pout)
                    else:
                        nc.vector.tensor_scalar_add(dst, pout, fcbr)

                # blk outputs
                nc.sync.dma_start(
                    out=x3_view[blk],
                    in_=x3sb.rearrange("f c q w -> f c (q w)"),
                )
                nc.sync.dma_start(
                    out=out_d[:, blk * BLK * P_ROWS:(blk + 1) * BLK * P_ROWS],
                    in_=outsb,
                )
    nc.compile()
    return nc


def _get_program(key):
    if key not in _CACHE:
        _CACHE[key] = _build_program(*key)
    return _CACHE[key]


def _host_inputs(x, A1, A2, A3, W1, b1, W2, b2, W3, b3, fcW, fcb):
    """Build per-core input maps. Returns (in_maps, key)."""
    cdt = np.float16 if COMPUTE_FP16 else np.float32
    A1, A2, A3 = (np.asarray(a, np.float32) for a in (A1, A2, A3))
    W1, W2, W3, fcW = (np.asarray(w, np.float32) for w in (W1, W2, W3, fcW))
    b1, b2, b3, fcb = (np.asarray(b, np.float32) for b in (b1, b2, b3, fcb))

    eye6 = np.eye(PACK, dtype=np.float32)
    a6 = np.ascontiguousarray(
        np.stack([np.kron(eye6, a) for a in (A1, A2, A3)])).astype(cdt)
    # W1 packed: [128, 256], chunk c cols = W1[c*128:(c+1)*128, :]
    w1p = np.ascontiguousarray(
        np.concatenate([W1[0:128, :], W1[128:256, :]], axis=1)).astype(cdt)
    w2 = np.ascontiguousarray(W2).astype(cdt)
    w3 = np.ascontiguousarray(W3).astype(cdt)
    fcwp = np.ascontiguousarray(
        np.concatenate([fcW[c * 128:(c + 1) * 128, :] for c in range(4)],
                       axis=1))

    has_b1, has_b2 = bool(b1.any()), bool(b2.any())
    has_b3, has_fcb = bool(b3.any()), bool(fcb.any())
    key = (has_b1, has_b2, has_b3, has_fcb)

    extra = {}
    if has_b1:
        extra["b1r"] = np.ascontiguousarray(
            np.broadcast_to(b1[None, :], (P_ROWS, D_MODEL)).astype(np.float32))
    if has_b2:
        extra["b2r"] = np.ascontiguousarray(
            np.broadcast_to(b2[None, :], (P_ROWS, D_MODEL)).astype(np.float32))
    if has_b3:
        # b3r[f, c] = b3[c*128+f]
        extra["b3r"] = np.ascontiguousarray(
            b3.reshape(4, 128).T.astype(np.float32))
    if has_fcb:
        extra["fcbr"] = np.ascontiguousarray(fcb[:, None])

    xs = np.asarray(x, np.float32).reshape(B_FULL, NUM_HAND, F_IN)
    in_maps = []
    for c in range(N_CORES):
        shard = xs[c * B_CORE:(c + 1) * B_CORE]
        shard = np.pad(shard, ((0, B_PAD - B_CORE), (0, 0), (0, 0)))
        m = {
            "x": np.ascontiguousarray(shard.reshape(ROWS_PAD, F_IN).astype(cdt)),
            "a6": a6, "w1p": w1p, "w2": w2, "w3": w3, "fcwp": fcwp,
        }
        m.update(extra)
        in_maps.append(m)
    return in_maps, key


def _run_cores(x, A1, A2, A3, W1, b1, W2, b2, W3, b3, fcW, fcb,
               run_fn, **run_kwargs):
    in_maps, key = _host_inputs(x, A1, A2, A3, W1, b1, W2, b2, W3, b3, fcW, fcb)
    nc = _get_program(key)
    res = run_fn(nc, in_maps, core_ids=list(range(N_CORES)), **run_kwargs)
    results = res.results if hasattr(res, "results") else res
    x3_parts, out_parts = [], []
    nrows = B_CORE * NUM_HAND
    for c in range(N_CORES):
        x3fm = np.asarray(results[c]["x3fm"])[:, :nrows]
        outfm = np.asarray(results[c]["outfm"])[:, :nrows]
        x3_parts.append(np.ascontiguousarray(x3fm.T).reshape(
            B_CORE, NUM_HAND, F_OUT))
        out_parts.append(np.ascontiguousarray(outfm.T).reshape(
            B_CORE, NUM_HAND, OUT_DIM))
    x3 = np.concatenate(x3_parts, axis=0)
    out = np.concatenate(out_parts, axis=0)
    return (x3, out)


def kernel(x, A1, A2, A3, W1, b1, W2, b2, W3, b3, fcW, fcb):
    from concourse.bass_utils import run_bass_kernel_spmd
    return _run_cores(x, A1, A2, A3, W1, b1, W2, b2, W3, b3, fcW, fcb,
                      run_bass_kernel_spmd)


# revision 6
# speedup vs baseline: 2.9005x; 1.2726x over previous
"""HandNet GCN kernel for 8x Trainium2 NeuronCores (Bass/Tile).

Network (per batch b of 8192, N=21 hand joints):
  x1 = relu(A @ x  @ W1 + b1)   [21,256] -> [21,128]
  x2 = relu(A @ x1 @ W2 + b2)   [21,128] -> [21,128]
  x3 = relu(A @ x2 @ W3 + b3)   [21,128] -> [21,512]
  out = x3 @ fcW + fcb          [21,512] -> [21,3]
Returns (x3, out).

Strategy: pure data parallelism over batch (1024 batches/core, padded to
1056). Six batches are packed per partition-group (6*21 = 126 rows), and
the adjacency is applied as kron(I6, A) [126x126]. Each layer runs as two
matmul families with alternating operand roles so the activation transpose
rides on LDWEIGHTS:
  A-mix : lhsT = act(node-major) [126, 128], rhs = A6 [126,126]
          -> out = (A @ X)^T feature-major in PSUM
  W-mul : lhsT = P(feature-major) [128, 128], rhs = W chunk -> node-major
  L3/FC : W3 chunks / fcW chunks as lhsT, activations as wide moving
          operand (N=504..512) -> feature-major.
Matmul operands are fp16 (fp32 PSUM accumulate); the final FC runs in fp32
off the fp32 x3 tiles. x3/out are produced feature-major and transposed on
the host; x is pre-cast to fp16 on the host (halves input DMA traffic).
"""

import numpy as np

NUM_HAND = 21
PACK = 6                      # batches per partition group
P_ROWS = PACK * NUM_HAND      # 126
N_CORES = 8
B_FULL = 8192
B_CORE = B_FULL // N_CORES    # 1024
GROUPS = 176                  # per core, multiple of 8 (=> B_PAD = 1056)
B_PAD = GROUPS * PACK         # 1056
ROWS_PAD = B_PAD * NUM_HAND   # 22176
F_IN = 256
D_MODEL = 128
F_OUT = 512
OUT_DIM = 3

BLK = 8                       # groups per outer block (x3/out DMA unit)
NBLK = GROUPS // BLK          # 22
QUAD = 4                      # groups per quad (inner compute unit)
W504 = QUAD * P_ROWS          # 504

COMPUTE_FP16 = True           # fp16 matmul operands (fp32 accumulate)

_CACHE = {}


def _build_program(has_b1, has_b2, has_b3, has_fcb):
    import concourse.bacc as bacc
    import concourse.mybir as mybir
    from concourse.tile import TileContext

    f32 = mybir.dt.float32
    cdt = mybir.dt.float16 if COMPUTE_FP16 else f32
    nc = bacc.Bacc("TRN2")

    x_d = nc.dram_tensor("x", [ROWS_PAD, F_IN], cdt, kind="ExternalInput")
    a6_d = nc.dram_tensor("a6", [3, P_ROWS, P_ROWS], cdt, kind="ExternalInput")
    w1_d = nc.dram_tensor("w1p", [D_MODEL, F_IN], cdt, kind="ExternalInput")
    w2_d = nc.dram_tensor("w2", [D_MODEL, D_MODEL], cdt, kind="ExternalInput")
    w3_d = nc.dram_tensor("w3", [D_MODEL, F_OUT], cdt, kind="ExternalInput")
    fcw_d = nc.dram_tensor("fcwp", [D_MODEL, 4 * OUT_DIM], cdt, kind="ExternalInput")
    x3_d = nc.dram_tensor("x3fm", [F_OUT, ROWS_PAD], cdt, kind="ExternalOutput")
    out_d = nc.dram_tensor("outfm", [OUT_DIM, ROWS_PAD], f32, kind="ExternalOutput")

    b1_d = b2_d = b3_d = fcb_d = None
    if has_b1:
        b1_d = nc.dram_tensor("b1r", [P_ROWS, D_MODEL], f32, kind="ExternalInput")
    if has_b2:
        b2_d = nc.dram_tensor("b2r", [P_ROWS, D_MODEL], f32, kind="ExternalInput")
    if has_b3:
        b3_d = nc.dram_tensor("b3r", [D_MODEL, 4], f32, kind="ExternalInput")
    if has_fcb:
        fcb_d = nc.dram_tensor("fcbr", [OUT_DIM, 1], f32, kind="ExternalInput")

    Relu = mybir.ActivationFunctionType.Relu
    Add = mybir.AluOpType.add

    # x rows grouped per blk: [NBLK, 126, BLK, 256]
    x_view = x_d.rearrange("(nb gb p) f -> nb p gb f", gb=BLK, p=P_ROWS)
    # x3 fm: row (c*128+f), col (blk, go, bn); per blk: [128, 4, 1008]
    x3_view = x3_d.rearrange("(c f) (nb w) -> nb f c w", f=D_MODEL, w=BLK * P_ROWS)

    with TileContext(nc) as tc:
        with (
            tc.tile_pool(name="consts", bufs=1) as consts,
            tc.tile_pool(name="xin", bufs=3) as xin_pool,
            tc.tile_pool(name="stage", bufs=3) as stage,
            tc.tile_pool(name="x3s", bufs=2) as x3s_pool,
            tc.tile_pool(name="outs", bufs=2) as outs_pool,
            tc.tile_pool(name="pP1", bufs=1, space="PSUM") as pP1,
            tc.tile_pool(name="pYP", bufs=3, space="PSUM") as pYP,
            tc.tile_pool(name="pX3", bufs=2, space="PSUM") as pX3,
            tc.tile_pool(name="pOut", bufs=1, space="PSUM") as pOut,
        ):
            # ---- constants ----
            a6 = []
            for l in range(3):
                t = consts.tile([P_ROWS, P_ROWS], cdt, tag=f"a6_{l}")
                nc.sync.dma_start(out=t, in_=a6_d[l])
                a6.append(t)
            w1 = consts.tile([D_MODEL, F_IN], cdt, tag="w1")
            nc.sync.dma_start(out=w1, in_=w1_d[:])
            w2 = consts.tile([D_MODEL, D_MODEL], cdt, tag="w2")
            nc.sync.dma_start(out=w2, in_=w2_d[:])
            w3 = consts.tile([D_MODEL, F_OUT], cdt, tag="w3")
            nc.sync.dma_start(out=w3, in_=w3_d[:])
            fcw = consts.tile([D_MODEL, 4 * OUT_DIM], cdt, tag="fcw")
            nc.sync.dma_start(out=fcw, in_=fcw_d[:])
            b1r = b2r = b3r = fcbr = None
            if has_b1:
                b1r = consts.tile([P_ROWS, D_MODEL], f32, tag="b1r")
                nc.sync.dma_start(out=b1r, in_=b1_d[:])
            if has_b2:
                b2r = consts.tile([P_ROWS, D_MODEL], f32, tag="b2r")
                nc.sync.dma_start(out=b2r, in_=b2_d[:])
            if has_b3:
                b3r = consts.tile([D_MODEL, 4], f32, tag="b3r")
                nc.sync.dma_start(out=b3r, in_=b3_d[:])
            if has_fcb:
                fcbr = consts.tile([OUT_DIM, 1], f32, tag="fcbr")
                nc.sync.dma_start(out=fcbr, in_=fcb_d[:])

            def relu_evict_nm(use_act, dst, src, brep):
                """dst = relu(src + b); node-major [126,4,128]; bias on free."""
                if brep is None:
                    if use_act:
                        nc.scalar.activation(dst, src, Relu)
                    else:
                        nc.vector.tensor_relu(dst, src)
                else:
                    for g in range(QUAD):
                        nc.vector.tensor_tensor(dst[:, g], src[:, g], brep, Add)
                    nc.scalar.activation(dst, dst, Relu)

            # ---- main loop ----
            for blk in range(NBLK):
                xt = xin_pool.tile([P_ROWS, BLK, F_IN], cdt, tag="xt")
                nc.sync.dma_start(out=xt, in_=x_view[blk])
                # x3 staging: [128, c=4, q=2, (g bn)=504] f32
                x3sb = x3s_pool.tile([D_MODEL, 4, 2, W504], cdt, tag="x3sb")
                outsb = outs_pool.tile([OUT_DIM, BLK * P_ROWS], f32, tag="outsb")

                for q in range(2):  # quads of 4 groups
                    # ---- L1 A-mix: 4 groups x 2 chunks -> p1 fm
                    p1 = pP1.tile([D_MODEL, 8, D_MODEL], f32, tag="pP1")
                    for g in range(QUAD):
                        xsl = xt[:, q * QUAD + g]
                        for c in range(2):
                            nc.tensor.matmul(
                                p1[:, g * 2 + c, :P_ROWS],
                                xsl[:, c * D_MODEL:(c + 1) * D_MODEL],
                                a6[0],
                            )
                    p1s = stage.tile([D_MODEL, 8, D_MODEL], cdt, tag="p1s")
                    nc.scalar.copy(p1s, p1)
                    # ---- L1 W-mul -> y1 node-major
                    y1 = pYP.tile([D_MODEL, QUAD, D_MODEL], f32, tag="pYP")
                    for g in range(QUAD):
                        for c in range(2):
                            nc.tensor.matmul(
                                y1[:, g],
                                p1s[:, g * 2 + c],
                                w1[:, c * D_MODEL:(c + 1) * D_MODEL],
                                start=(c == 0),
                                stop=(c == 1),
                            )
                    h2 = stage.tile([P_ROWS, QUAD, D_MODEL], cdt, tag="h2")
                    relu_evict_nm(True, h2, y1[:P_ROWS], b1r)
                    # ---- L2
                    p2 = pYP.tile([D_MODEL, QUAD, D_MODEL], f32, tag="pYP")
                    for g in range(QUAD):
                        nc.tensor.matmul(p2[:, g, :P_ROWS], h2[:, g], a6[1])
                    p2s = stage.tile([D_MODEL, QUAD, D_MODEL], cdt, tag="p2s")
                    nc.scalar.copy(p2s, p2)
                    y2 = pYP.tile([D_MODEL, QUAD, D_MODEL], f32, tag="pYP")
                    for g in range(QUAD):
                        nc.tensor.matmul(y2[:, g], p2s[:, g], w2)
                    h3 = stage.tile([P_ROWS, QUAD, D_MODEL], cdt, tag="h3")
                    relu_evict_nm(False, h3, y2[:P_ROWS], b2r)
                    # ---- L3 A-mix
                    p3 = pYP.tile([D_MODEL, QUAD, D_MODEL], f32, tag="pYP")
                    for g in range(QUAD):
                        nc.tensor.matmul(p3[:, g, :P_ROWS], h3[:, g], a6[2])
                    p3s = stage.tile([D_MODEL, QUAD, D_MODEL], cdt, tag="p3s")
                    nc.vector.tensor_copy(p3s, p3)
                    # ---- L3 W-mul fm, one wide matmul per W3 chunk
                    p3w = p3s.rearrange("f g d -> f (g d)")  # [128, 512]
                    for c in range(4):
                        px3 = pX3.tile([D_MODEL, QUAD, D_MODEL], f32, tag="pX3")
                        nc.tensor.matmul(
                            px3.rearrange("f g d -> f (g d)"),
                            w3[:, c * D_MODEL:(c + 1) * D_MODEL],
                            p3w,
                        )
                        # evict valid cols: [128, 4, :126] -> x3sb[:, c, q, :]
                        dst = x3sb[:, c, q].rearrange("f (g bn) -> f g bn",
                                                      g=QUAD)
                        src = px3[:, :, :P_ROWS]
                        if b3r is None:
                            if c % 2 == 0:
                                nc.scalar.activation(dst, src, Relu)
                            else:
                                nc.vector.tensor_relu(dst, src)
                        else:
                            nc.scalar.activation(dst, src, Relu,
                                                 bias=b3r[:, c:c + 1])
                    # ---- FC (fp32): fcW chunks as lhsT, accumulate over c
                    pout = pOut.tile([OUT_DIM, W504], f32, tag="pOut")
                    for c in range(4):
                        nc.tensor.matmul(
                            pout,
                            fcw[:, c * OUT_DIM:(c + 1) * OUT_DIM],
                            x3sb[:, c, q],
                            start=(c == 0),
                            stop=(c == 3),
                        )
                    dst = outsb[:, q * W504:(q + 1) * W504]
                    if fcbr is None:
                        nc.vector.tensor_copy(dst, pout)
                    else:
                        nc.vector.tensor_scalar_add(dst, pout, fcbr)

                # blk outputs
                nc.sync.dma_start(
                    out=x3_view[blk],
                    in_=x3sb.rearrange("f c q w -> f c (q w)"),
                )
                nc.sync.dma_start(
                    out=out_d[:, blk * BLK * P_ROWS:(blk + 1) * BLK * P_ROWS],
                    in_=outsb,
                )
    nc.compile()
    return nc


def _get_program(key):
    if key not in _CACHE:
        _CACHE[key] = _build_program(*key)
    return _CACHE[key]


def _host_inputs(x, A1, A2, A3, W1, b1, W2, b2, W3, b3, fcW, fcb):
    """Build per-core input maps. Returns (in_maps, key)."""
    cdt = np.float16 if COMPUTE_FP16 else np.float32
    A1, A2, A3 = (np.asarray(a, np.float32) for a in (A1, A2, A3))
    W1, W2, W3, fcW = (np.asarray(w, np.float32) for w in (W1, W2, W3, fcW))
    b1, b2, b3, fcb = (np.asarray(b, np.float32) for b in (b1, b2, b3, fcb))

    eye6 = np.eye(PACK, dtype=np.float32)
    a6 = np.ascontiguousarray(
        np.stack([np.kron(eye6, a) for a in (A1, A2, A3)])).astype(cdt)
    # W1 packed: [128, 256], chunk c cols = W1[c*128:(c+1)*128, :]
    w1p = np.ascontiguousarray(
        np.concatenate([W1[0:128, :], W1[128:256, :]], axis=1)).astype(cdt)
    w2 = np.ascontiguousarray(W2).astype(cdt)
    w3 = np.ascontiguousarray(W3).astype(cdt)
    fcwp = np.ascontiguousarray(
        np.concatenate([fcW[c * 128:(c + 1) * 128, :] for c in range(4)],
                       axis=1)).astype(cdt)

    has_b1, has_b2 = bool(b1.any()), bool(b2.any())
    has_b3, has_fcb = bool(b3.any()), bool(fcb.any())
    key = (has_b1, has_b2, has_b3, has_fcb)

    extra = {}
    if has_b1:
        extra["b1r"] = np.ascontiguousarray(
            np.broadcast_to(b1[None, :], (P_ROWS, D_MODEL)).astype(np.float32))
    if has_b2:
        extra["b2r"] = np.ascontiguousarray(
            np.broadcast_to(b2[None, :], (P_ROWS, D_MODEL)).astype(np.float32))
    if has_b3:
        # b3r[f, c] = b3[c*128+f]
        extra["b3r"] = np.ascontiguousarray(
            b3.reshape(4, 128).T.astype(np.float32))
    if has_fcb:
        extra["fcbr"] = np.ascontiguousarray(fcb[:, None])

    xs = np.asarray(x, np.float32).reshape(B_FULL, NUM_HAND, F_IN)
    in_maps = []
    for c in range(N_CORES):
        shard = xs[c * B_CORE:(c + 1) * B_CORE]
        shard = np.pad(shard, ((0, B_PAD - B_CORE), (0, 0), (0, 0)))
        m = {
            "x": np.ascontiguousarray(shard.reshape(ROWS_PAD, F_IN).astype(cdt)),
            "a6": a6, "w1p": w1p, "w2": w2, "w3": w3, "fcwp": fcwp,
        }
        m.update(extra)
        in_maps.append(m)
    return in_maps, key


def _run_cores(x, A1, A2, A3, W1, b1, W2, b2, W3, b3, fcW, fcb,
               run_fn, **run_kwargs):
    in_maps, key = _host_inputs(x, A1, A2, A3, W1, b1, W2, b2, W3, b3, fcW, fcb)
    nc = _get_program(key)
    res = run_fn(nc, in_maps, core_ids=list(range(N_CORES)), **run_kwargs)
    results = res.results if hasattr(res, "results") else res
    x3_parts, out_parts = [], []
    nrows = B_CORE * NUM_HAND
    for c in range(N_CORES):
        x3fm = np.asarray(results[c]["x3fm"])[:, :nrows].astype(np.float32)
        outfm = np.asarray(results[c]["outfm"])[:, :nrows]
        x3_parts.append(np.ascontiguousarray(x3fm.T).reshape(
            B_CORE, NUM_HAND, F_OUT))
        out_parts.append(np.ascontiguousarray(outfm.T).reshape(
            B_CORE, NUM_HAND, OUT_DIM))
    x3 = np.concatenate(x3_parts, axis=0)
    out = np.concatenate(out_parts, axis=0)
    return (x3, out)


def kernel(x, A1, A2, A3, W1, b1, W2, b2, W3, b3, fcW, fcb):
    from concourse.bass_utils import run_bass_kernel_spmd
    return _run_cores(x, A1, A2, A3, W1, b1, W2, b2, W3, b3, fcW, fcb,
                      run_bass_kernel_spmd)
